# revision 1
# baseline (speedup 1.0000x reference)
"""MoE transformer layer (soft-routed) on 8 TRN2 NeuronCores.

Strategy: pure batch-data-parallel. B == n_cores == 8, so each core runs the
full layer for one batch element; no collectives. All activations live in
transposed [feature, seq] layout so every matmul contracts over the partition
dim. Weights are transposed/folded on the host (norm weights folded into the
following projection, attention scale folded into q, per-core routing weight
folded into the expert down-projections, biases folded into rows used by K=1
bias matmuls). RoPE partner-swap is a 128x128 permutation matmul on the PE.
Matmuls run in float32r (full-rate fp32 replication mode).
"""
import sys, os

for _p in ("/opt/trn_rl_repo", "/root/.axon_site/_ro/trn_rl_repo"):
    if os.path.isdir(_p) and _p not in sys.path:
        sys.path.insert(0, _p)

import numpy as np
import concourse.bacc as bacc
import concourse.mybir as mybir
from concourse import bass_utils
from concourse.tile import TileContext

f32 = mybir.dt.float32
f32r = mybir.dt.float32r
AF = mybir.ActivationFunctionType
OP = mybir.AluOpType

FULL = dict(D=1024, S=1024, NH=16, SH=4096, RH=1024, NE=8, NCORES=8)
EPS = 1e-5
ROPE_SCALES = (1.0, 1.0, 1.0, 0.2)
ROPE_BASE = 10000.0


# ---------------------------------------------------------------- host prep

def host_prep(cfg, src, centroids, routing_weights, qkv_w, qkv_b, out_w, out_b,
              norm1_w, norm2_w, gamma_1, gamma_2,
              sh_wg, sh_bg, sh_wu, sh_bu, sh_wd, sh_bd,
              r_wg, r_bg, r_wu, r_bu, r_wd, r_bd):
    """Returns list of per-core input dicts (one batch element per core)."""
    D, S, NH, SH, RH, NE = cfg["D"], cfg["S"], cfg["NH"], cfg["SH"], cfg["RH"], cfg["NE"]
    DH = D // NH
    KD, SHT, RHT = D // 128, SH // 128, RH // 128
    B = src.shape[0]
    c = lambda a: np.ascontiguousarray(a, dtype=np.float32)
    scale = 1.0 / np.sqrt(DH)

    qkvT = (qkv_w.T * norm1_w[:, None]).copy()
    qkvT[:, :D] *= scale
    qkv_b_row = np.array(qkv_b, np.float32).copy()
    qkv_b_row[:D] *= scale

    shared = {
        "qkvT_w": c(qkvT),
        "out_wT": c(out_w.T),
        "g1_col": c(np.asarray(gamma_1).reshape(KD, 128).T),
        "g2_col": c(np.asarray(gamma_2).reshape(KD, 128).T),
        "sh_wgT": c(sh_wg.T * norm2_w[:, None]),
        "sh_wuT": c(sh_wu.T * norm2_w[:, None]),
        "sh_wdT": c(sh_wd.T),
        "sh_bg_col": c(np.asarray(sh_bg).reshape(SHT, 128).T),
        "sh_bu_col": c(np.asarray(sh_bu).reshape(SHT, 128).T),
        "r_wgT": c(np.stack([r_wg[e].T * norm2_w[:, None] for e in range(NE)])),
        "r_wuT": c(np.stack([r_wu[e].T * norm2_w[:, None] for e in range(NE)])),
        "r_bg_col": c(np.asarray(r_bg).reshape(NE, RHT, 128).transpose(2, 0, 1).reshape(128, NE * RHT)),
        "r_bu_col": c(np.asarray(r_bu).reshape(NE, RHT, 128).transpose(2, 0, 1).reshape(128, NE * RHT)),
    }
    perm = np.zeros((128, 128), np.float32)
    perm[np.arange(128) ^ 8, np.arange(128)] = 1.0
    shared["perm"] = perm

    # qkv_b / out_b / sh_bd / r_bd are all-zero in this problem's
    # setup_inputs; the kernel skips those bias adds.
    assert not np.any(qkv_b) and not np.any(out_b), "nonzero qkv/out bias unsupported"
    assert not np.any(sh_bd) and not np.any(r_bd), "nonzero down-proj bias unsupported"

    # rope tables: row p handles head-local feature dh = p % DH
    p = np.arange(128)
    dh = p % DH
    d_axis = DH // 4
    half = d_axis // 2
    a_idx = dh // d_axis
    j = dh % d_axis
    f = j % half
    sign = np.where(j < half, -1.0, 1.0).astype(np.float32)
    inv_freq = (1.0 / (ROPE_BASE ** (np.arange(half) / half))).astype(np.float32)

    in_maps = []
    for b in range(B):
        m = dict(shared)
        m["srcT"] = c(np.asarray(src[b]).T)
        pos = np.asarray(centroids[b])[:, a_idx] * np.asarray(ROPE_SCALES, np.float32)[a_idx]
        ang = (pos * inv_freq[f][None, :]).T.astype(np.float32)        # [128, S]
        m["cosT"] = c(np.cos(ang))
        m["sinT"] = c(sign[:, None] * np.sin(ang))
        rw = np.asarray(routing_weights[b], np.float32)
        m["r_wdTs"] = c(np.stack([(rw[e] * r_wd[e]).T for e in range(NE)]))
        in_maps.append(m)
    return in_maps


# ---------------------------------------------------------------- device build

def build_nc(cfg, reps=1):
    D, S, NH, SH, RH, NE = cfg["D"], cfg["S"], cfg["NH"], cfg["SH"], cfg["RH"], cfg["NE"]
    DH = D // NH
    assert DH == 64 and D % 128 == 0 and S % 128 == 0 and SH % 128 == 0 and RH % 128 == 0
    KD, ST, SHT, RHT = D // 128, S // 128, SH // 128, RH // 128
    CH = min(512, S)
    NCH = S // CH
    HP = NH // 2          # head pairs
    VS = DH + 1           # v stride per head (64 data + ones col)
    HPC = min(NH, 512 // DH)   # heads per v-projection chunk
    VW = HPC * DH              # v chunk width

    nc = bacc.Bacc("TRN2", target_bir_lowering=False, debug=False)

    def din(name, shape):
        return nc.dram_tensor(name, list(shape), f32, kind="ExternalInput")

    srcT_d = din("srcT", (D, S))
    cosT_d = din("cosT", (128, S))
    sinT_d = din("sinT", (128, S))
    perm_d = din("perm", (128, 128))
    qkvT_d = din("qkvT_w", (D, 3 * D))
    outwT_d = din("out_wT", (D, D))
    g1_d = din("g1_col", (128, KD))
    g2_d = din("g2_col", (128, KD))
    shwg_d = din("sh_wgT", (D, SH))
    shwu_d = din("sh_wuT", (D, SH))
    shwd_d = din("sh_wdT", (SH, D))
    shbg_d = din("sh_bg_col", (128, SHT))
    shbu_d = din("sh_bu_col", (128, SHT))
    rwg_d = din("r_wgT", (NE, D, RH))
    rwu_d = din("r_wuT", (NE, D, RH))
    rwd_d = din("r_wdTs", (NE, RH, D))
    rbg_d = din("r_bg_col", (128, NE * RHT))
    rbu_d = din("r_bu_col", (128, NE * RHT))
    outT_d = nc.dram_tensor("outT", [D, S], f32, kind="ExternalOutput")
    DBG = cfg.get("DBG", False)
    if DBG:
        dbg = {n: nc.dram_tensor(f"dbg_{n}", [D, S], f32, kind="ExternalOutput")
               for n in ("xnT", "attn", "x1T", "xn2T", "ff")}
        dbg_rot = nc.dram_tensor("dbg_rot", [256, S], f32, kind="ExternalOutput")

    def load_w(pool, dram_ap, m0, mw, ktot, tag, bufs, name):
        """Stream weight cols [m0, m0+mw) of a [ktot, *] matrix into a
        [128, (ktot/128)*mw] tile; k-tile kt lives at [:, kt*mw:(kt+1)*mw]."""
        kt = ktot // 128
        t = pool.tile([128, kt * mw], f32r, name=name, tag=tag, bufs=bufs)
        src = dram_ap.rearrange("(k p) e -> p k e", p=128)[:, :, m0:m0 + mw]
        nc.sync.dma_start(t[:].rearrange("p (k m) -> p k m", k=kt), src.bitcast(f32r))
        return t

    with TileContext(nc) as tc:
      for rep_i in range(reps):
          cpool = tc.alloc_tile_pool(name=f"const{rep_i}", bufs=1)

          onesf = cpool.tile([128, 32], f32, name="onesf", tag="onesf")
          nc.vector.memset(onesf[:], 1.0)
          ones_row_f = cpool.tile([1, 512], f32, name="ones_row_f", tag="ones_row_f")
          nc.vector.memset(ones_row_f[:], 1.0)
          ones_row = cpool.tile([1, 512], f32r, name="ones_row", tag="ones_row")
          nc.vector.tensor_copy(ones_row[:], ones_row_f[:])
          ones_col = cpool.tile([128, 1], f32r, name="ones_col", tag="ones_col")
          nc.vector.tensor_copy(ones_col[:], onesf[:, 0:1])
          eps1 = cpool.tile([1, 1], f32, name="eps1", tag="eps1")
          nc.vector.memset(eps1[:], EPS)
          permt = cpool.tile([128, 128], f32r, name="permt", tag="permt")
          nc.sync.dma_start(permt[:], perm_d.ap().bitcast(f32r))
          cosT = cpool.tile([128, S], f32, name="cosT", tag="cosT")
          nc.sync.dma_start(cosT[:], cosT_d.ap())
          sinT = cpool.tile([128, S], f32, name="sinT", tag="sinT")
          nc.sync.dma_start(sinT[:], sinT_d.ap())
          g1c = cpool.tile([128, KD], f32, name="g1c", tag="g1c")
          nc.sync.dma_start(g1c[:], g1_d.ap())
          g2c = cpool.tile([128, KD], f32, name="g2c", tag="g2c")
          nc.sync.dma_start(g2c[:], g2_d.ap())
          shbg = cpool.tile([128, SHT], f32, name="shbg", tag="shbg")
          nc.sync.dma_start(shbg[:], shbg_d.ap())
          shbu = cpool.tile([128, SHT], f32, name="shbu", tag="shbu")
          nc.sync.dma_start(shbu[:], shbu_d.ap())
          rbg = cpool.tile([128, NE * RHT], f32, name="rbg", tag="rbg")
          nc.sync.dma_start(rbg[:], rbg_d.ap())
          rbu = cpool.tile([128, NE * RHT], f32, name="rbu", tag="rbu")
          nc.sync.dma_start(rbu[:], rbu_d.ap())

          ffpool = tc.alloc_tile_pool(name=f"ffpool{rep_i}", bufs=1)    # ff (D..end)
          ab_pool = tc.alloc_tile_pool(name=f"ab{rep_i}", bufs=1)       # slotA: xnT->x1T, slotB: attn->xn2T
          xnT = [ab_pool.tile([128, S], f32r, name=f"xnT{kt}", tag=f"slotA{kt}")
                 for kt in range(KD)]
          attn = [ab_pool.tile([128, S], f32r, name=f"attn{kt}", tag=f"slotB{kt}")
                  for kt in range(KD)]

          # ---------------- phase A: rms norm 1 -> xnT (= src * rstd) -----
          srcA = tc.alloc_tile_pool(name=f"srcA{rep_i}", bufs=1)
          psA = tc.alloc_tile_pool(name=f"psA{rep_i}", bufs=1, space="PSUM")
          sqA = tc.alloc_tile_pool(name=f"sqA{rep_i}", bufs=1)
          srcT = []
          for kt in range(KD):
              t = srcA.tile([128, S], f32, name=f"srcT{kt}", tag=f"srcT{kt}")
              nc.sync.dma_start(t[:], srcT_d.ap()[kt * 128:(kt + 1) * 128, :])
              srcT.append(t)
          for c in range(NCH):
              cs = slice(c * CH, (c + 1) * CH)
              vrow_ps = psA.tile([1, CH], f32, name="vrow_ps", tag="vrow", bufs=2)
              for kt in range(KD):
                  sq = sqA.tile([128, CH], f32r, name="sq", tag="sq", bufs=3)
                  nc.vector.tensor_mul(sq[:], srcT[kt][:, cs], srcT[kt][:, cs])
                  nc.tensor.matmul(vrow_ps[:], ones_col[:], sq[:],
                                   start=(kt == 0), stop=(kt == KD - 1))
              srr = sqA.tile([1, 2 * CH], f32r, name="srr", tag="srr", bufs=2)
              srow = srr[0:1, 0:CH]
              rrow = srr[0:1, CH:2 * CH]
              nc.scalar.activation(srow, vrow_ps[:], AF.Sqrt,
                                   bias=eps1[:1, 0:1], scale=1.0 / D)
              with nc.allow_low_precision(reason="rstd fp32r rounding ok"):
                  nc.vector.reciprocal(rrow, srow)
              bc = psA.tile([128, CH], f32, name="bcA", tag="bcA", bufs=2)
              nc.tensor.matmul(bc[:], ones_row[:1, 0:128], rrow, start=True, stop=True)
              for kt in range(KD):
                  nc.vector.tensor_mul(xnT[kt][:, cs], srcT[kt][:, cs], bc[:])
          if DBG:
              for kt in range(KD):
                  nc.sync.dma_start(dbg["xnT"].ap()[kt * 128:(kt + 1) * 128, :],
                                    xnT[kt][:].bitcast(f32))
          sqA.release()
          psA.release()
          srcA.release()

          # ---------------- phase B: attention --------------------------
          wB = tc.alloc_tile_pool(name=f"wB{rep_i}", bufs=1)
          qkB = tc.alloc_tile_pool(name=f"qkB{rep_i}", bufs=1)
          vB = tc.alloc_tile_pool(name=f"vB{rep_i}", bufs=1)
          psAcc = tc.alloc_tile_pool(name=f"psAcc{rep_i}", bufs=2, space="PSUM")
          psMix = tc.alloc_tile_pool(name=f"psMix{rep_i}", bufs=1, space="PSUM")

          # ---- v in natural [s, heads*VS] layout, ones col appended --
          v_sb = [vB.tile([128, NH * VS], f32r, name=f"v{st}", tag=f"v{st}")
                  for st in range(ST)]
          for vb in range(NH // HPC):
              wv = load_w(wB, qkvT_d.ap(), 2 * D + vb * VW, VW, D,
                          tag="wv", bufs=1, name=f"wv{vb}")
              for st in range(ST):
                  pv = psAcc.tile([128, VW], f32, name="pv", tag="acc")
                  for kt in range(KD):
                      nc.tensor.matmul(pv[:], xnT[kt][:, st * 128:(st + 1) * 128],
                                       wv[:, kt * VW:(kt + 1) * VW],
                                       start=(kt == 0), stop=(kt == KD - 1))
                  dst = v_sb[st][:, vb * HPC * VS:(vb + 1) * HPC * VS] \
                      .rearrange("p (h c) -> p h c", h=HPC)[:, :, 0:DH]
                  nc.scalar.activation(dst, pv[:].rearrange("p (h c) -> p h c", c=DH),
                                       AF.Copy)
          for st in range(ST):
              oc = v_sb[st][:].rearrange("p (h c) -> p h c", c=VS)[:, :, DH:DH + 1]
              nc.vector.tensor_copy(oc, onesf[:, 0:NH].rearrange("p (h c) -> p h c", c=1))

          # ---- per head pair: qk proj, rope, scores, av ------------
          for hp in range(HP):
              wq = load_w(wB, qkvT_d.ap(), hp * 128, 128, D, tag="wq", bufs=1, name=f"wq{hp}")
              wk = load_w(wB, qkvT_d.ap(), D + hp * 128, 128, D, tag="wk", bufs=1, name=f"wk{hp}")
              rots = {}
              for which, wt, coff in (("q", wq, hp * 128), ("k", wk, D + hp * 128)):
                  rot = qkB.tile([128, S], f32r, name=f"{which}rot", tag=f"{which}rot", bufs=2)
                  for c in range(NCH):
                      cs = slice(c * CH, (c + 1) * CH)
                      pq = psAcc.tile([128, CH], f32, name="pq", tag="acc")
                      for kt in range(KD):
                          nc.tensor.matmul(pq[:], wt[:, kt * 128:(kt + 1) * 128],
                                           xnT[kt][:, cs], start=(kt == 0), stop=(kt == KD - 1))
                      sb = qkB.tile([128, CH], f32r, name="qksb", tag="qksb", bufs=2)
                      nc.vector.tensor_copy(sb[:], pq[:])
                      psw = psMix.tile([128, CH], f32, name="psw", tag="swap", bufs=1)
                      nc.tensor.matmul(psw[:], permt[:], sb[:], start=True, stop=True)
                      nc.vector.tensor_mul(rot[:, cs], sb[:], cosT[:, cs])
                      t2 = qkB.tile([128, CH], f32, name="ropet2", tag="ropet2", bufs=2)
                      nc.vector.tensor_mul(t2[:], psw[:], sinT[:, cs])
                      nc.vector.tensor_add(rot[:, cs], rot[:, cs], t2[:])
                  if DBG and hp == 0:
                      nc.sync.dma_start(
                          dbg_rot.ap()[(0 if which == "q" else 128):(128 if which == "q" else 256), :],
                          rot[:].bitcast(f32))
                  rots[which] = rot

              # both heads of the pair interleaved: their K=64 scores matmuls
              # sit on distinct PE row-groups (base partition 0 / 64) and pack
              for c in range(NCH):
                  cs = slice(c * CH, (c + 1) * CH)
                  pavs = [psMix.tile([DH + 1, CH], f32, name=f"pav{hh}", tag=f"av{hh}", bufs=1)
                          for hh in range(2)]
                  for skt in range(ST):
                      exs = []
                      for hh in range(2):
                          hs = slice(64 * hh, 64 * hh + 64)
                          psc = psMix.tile([128, CH], f32, name="psc", tag="sc", bufs=3)
                          nc.tensor.matmul(psc[:], rots["k"][hs, skt * 128:(skt + 1) * 128],
                                           rots["q"][hs, cs], start=True, stop=True)
                          ex = qkB.tile([128, CH], f32r, name="ex", tag="ex", bufs=3)
                          nc.scalar.activation(ex[:], psc[:], AF.Exp)
                          exs.append(ex)
                      for hh in range(2):
                          h = 2 * hp + hh
                          nc.tensor.matmul(pavs[hh][:], v_sb[skt][:, h * VS:h * VS + DH + 1],
                                           exs[hh][:], start=(skt == 0), stop=(skt == ST - 1))
                  for hh in range(2):
                      pav = pavs[hh]
                      rec = qkB.tile([1, CH], f32r, name="rec", tag="rec", bufs=1)
                      with nc.allow_low_precision(reason="softmax denom recip"):
                          nc.vector.reciprocal(rec[:], pav[DH:DH + 1, :])
                      pbc = psMix.tile([64, CH], f32, name="pbc", tag="swap", bufs=1)
                      nc.tensor.matmul(pbc[:], ones_row[:1, 0:64], rec[:], start=True, stop=True)
                      bcs = qkB.tile([64, CH], f32, name="bcs", tag="bcs", bufs=1)
                      nc.vector.tensor_copy(bcs[:], pbc[:])
                      at = attn[hp][64 * hh:64 * hh + 64, cs]
                      nc.vector.tensor_mul(at, pav[0:DH, :], bcs[:])
          if DBG:
              for kt in range(KD):
                  nc.sync.dma_start(dbg["attn"].ap()[kt * 128:(kt + 1) * 128, :],
                                    attn[kt][:].bitcast(f32))
          psMix.release()
          psAcc.release()
          vB.release()
          qkB.release()
          wB.release()

          # ---------------- phase C: out proj + residual + norm2 ---------
          x1T = [ab_pool.tile([128, S], f32, name=f"x1T{kt}", tag=f"slotA{kt}")
                 for kt in range(KD)]
          wC = tc.alloc_tile_pool(name=f"wC{rep_i}", bufs=1)
          srcC = tc.alloc_tile_pool(name=f"srcC{rep_i}", bufs=1)
          psC = tc.alloc_tile_pool(name=f"psC{rep_i}", bufs=1, space="PSUM")
          for et in range(KD):
              wo = load_w(wC, outwT_d.ap(), et * 128, 128, D, tag="wo", bufs=2, name=f"wo{et}")
              sc_t = srcC.tile([128, S], f32, name="srcCt", tag="srcCt", bufs=2)
              nc.sync.dma_start(sc_t[:], srcT_d.ap()[et * 128:(et + 1) * 128, :])
              for c in range(NCH):
                  cs = slice(c * CH, (c + 1) * CH)
                  po = psC.tile([128, CH], f32, name="po", tag="acc", bufs=2)
                  for kt in range(KD):
                      nc.tensor.matmul(po[:], wo[:, kt * 128:(kt + 1) * 128],
                                       attn[kt][:, cs], start=(kt == 0), stop=(kt == KD - 1))
                  nc.vector.scalar_tensor_tensor(x1T[et][:, cs], po[:],
                                                 g1c[:, et:et + 1], sc_t[:, cs],
                                                 op0=OP.mult, op1=OP.add)
          if DBG:
              for kt in range(KD):
                  nc.sync.dma_start(dbg["x1T"].ap()[kt * 128:(kt + 1) * 128, :], x1T[kt][:])
          psC.release()
          srcC.release()
          wC.release()

          # norm2 -> xn2T
          xn2T = [ab_pool.tile([128, S], f32r, name=f"xn2T{kt}", tag=f"slotB{kt}")
                  for kt in range(KD)]
          psN2 = tc.alloc_tile_pool(name=f"psN2{rep_i}", bufs=1, space="PSUM")
          sqN2 = tc.alloc_tile_pool(name=f"sqN2{rep_i}", bufs=1)
          for c in range(NCH):
              cs = slice(c * CH, (c + 1) * CH)
              vrow2 = psN2.tile([1, CH], f32, name="vrow2", tag="vrow2", bufs=2)
              for kt in range(KD):
                  sq2 = sqN2.tile([128, CH], f32r, name="sq2", tag="sq2", bufs=3)
                  nc.vector.tensor_mul(sq2[:], x1T[kt][:, cs], x1T[kt][:, cs])
                  nc.tensor.matmul(vrow2[:], ones_col[:], sq2[:],
                                   start=(kt == 0), stop=(kt == KD - 1))
              srr2 = sqN2.tile([1, 2 * CH], f32r, name="srr2", tag="srr2", bufs=2)
              srow2 = srr2[0:1, 0:CH]
              rrow2 = srr2[0:1, CH:2 * CH]
              nc.scalar.activation(srow2, vrow2[:], AF.Sqrt,
                                   bias=eps1[:1, 0:1], scale=1.0 / D)
              with nc.allow_low_precision(reason="rstd fp32r rounding ok"):
                  nc.vector.reciprocal(rrow2, srow2)
              bc2 = psN2.tile([128, CH], f32, name="bc2", tag="bc2", bufs=2)
              nc.tensor.matmul(bc2[:], ones_row[:1, 0:128], rrow2, start=True, stop=True)
              for kt in range(KD):
                  nc.vector.tensor_mul(xn2T[kt][:, cs], x1T[kt][:, cs], bc2[:])
          if DBG:
              for kt in range(KD):
                  nc.sync.dma_start(dbg["xn2T"].ap()[kt * 128:(kt + 1) * 128, :],
                                    xn2T[kt][:].bitcast(f32))
          sqN2.release()
          psN2.release()

          # ---------------- phase D: FFN (shared + experts, fused) -------
          ff = [ffpool.tile([128, S], f32, name=f"ff{kt}", tag=f"ff{kt}")
                for kt in range(KD)]
          GRP = 8  # h-tiles per down-projection group
          wD = tc.alloc_tile_pool(name=f"wD{rep_i}", bufs=1)
          hD = tc.alloc_tile_pool(name=f"hD{rep_i}", bufs=1)
          psD = tc.alloc_tile_pool(name=f"psD{rep_i}", bufs=1, space="PSUM")
          hbuf = [hD.tile([128, S], f32r, name=f"hb{i}", tag=f"hb{i}")
                  for i in range(GRP)]
          first_group = [True]

          def gate_up(wgT_ap, wuT_ap, bg_col, bg_off, bu_col, bu_off, ht, slot):
              """hbuf[slot] = silu(xn2 @ wg + bg) * (xn2 @ wu + bu)"""
              wg = load_w(wD, wgT_ap, ht * 128, 128, D, tag="wg", bufs=3, name="wg")
              wu = load_w(wD, wuT_ap, ht * 128, 128, D, tag="wu", bufs=3, name="wu")
              for c in range(NCH):
                  cs = slice(c * CH, (c + 1) * CH)
                  pg = psD.tile([128, CH], f32, name="pg", tag="pg", bufs=2)
                  pu = psD.tile([128, CH], f32, name="pu", tag="pu", bufs=2)
                  for kt in range(KD):
                      nc.tensor.matmul(pg[:], wg[:, kt * 128:(kt + 1) * 128],
                                       xn2T[kt][:, cs], start=(kt == 0), stop=(kt == KD - 1))
                  for kt in range(KD):
                      nc.tensor.matmul(pu[:], wu[:, kt * 128:(kt + 1) * 128],
                                       xn2T[kt][:, cs], start=(kt == 0), stop=(kt == KD - 1))
                  sg = hD.tile([128, CH], f32r, name="sg", tag="sg", bufs=2)
                  nc.scalar.activation(sg[:], pg[:], AF.Silu,
                                       bias=bg_col[:, bg_off + ht:bg_off + ht + 1])
                  nc.vector.scalar_tensor_tensor(hbuf[slot][:, cs], pu[:],
                                                 bu_col[:, bu_off + ht:bu_off + ht + 1],
                                                 sg[:], op0=OP.add, op1=OP.mult)

          def down(wdT_ap, nht):
              """ff += down-projection of hbuf[0:nht]"""
              for dt in range(KD):
                  wd = load_w(wD, wdT_ap, dt * 128, 128, nht * 128,
                              tag="wd", bufs=2, name="wd")
                  for c in range(NCH):
                      cs = slice(c * CH, (c + 1) * CH)
                      pd = psD.tile([128, CH], f32, name="pd", tag="pd", bufs=2)
                      for i in range(nht):
                          nc.tensor.matmul(pd[:], wd[:, i * 128:(i + 1) * 128],
                                           hbuf[i][:, cs], start=(i == 0),
                                           stop=(i == nht - 1))
                      if first_group[0]:
                          nc.vector.tensor_copy(ff[dt][:, cs], pd[:])
                      else:
                          nc.vector.tensor_add(ff[dt][:, cs], ff[dt][:, cs], pd[:])
              first_group[0] = False

          for g0 in range(0, SHT, GRP):
              n = min(GRP, SHT - g0)
              for i in range(n):
                  gate_up(shwg_d.ap(), shwu_d.ap(), shbg, 0, shbu, 0, g0 + i, i)
              down(shwd_d.ap()[g0 * 128:(g0 + n) * 128, :], n)
          for e in range(NE):
              for i in range(RHT):
                  gate_up(rwg_d.ap()[e], rwu_d.ap()[e], rbg, e * RHT, rbu, e * RHT, i, i)
              down(rwd_d.ap()[e], RHT)
          if DBG:
              for kt in range(KD):
                  nc.sync.dma_start(dbg["ff"].ap()[kt * 128:(kt + 1) * 128, :], ff[kt][:])
          psD.release()
          hD.release()
          wD.release()

          # ---------------- final: out = x1 + g2 * ff --------------------
          outP = tc.alloc_tile_pool(name=f"outP{rep_i}", bufs=1)
          for dt in range(KD):
              ot = outP.tile([128, S], f32, name="ot", tag="ot", bufs=2)
              nc.vector.scalar_tensor_tensor(ot[:], ff[dt][:], g2c[:, dt:dt + 1],
                                             x1T[dt][:], op0=OP.mult, op1=OP.add)
              nc.sync.dma_start(outT_d.ap()[dt * 128:(dt + 1) * 128, :], ot[:])
          outP.release()
          ab_pool.release()
          ffpool.release()
          cpool.release()

    nc.compile()
    return nc


# ---------------------------------------------------------------- entry point

_CACHE = {}

_IN_ORDER = ["src", "centroids", "routing_weights", "qkv_w", "qkv_b", "out_w",
             "out_b", "norm1_w", "norm2_w", "gamma_1", "gamma_2",
             "sh_wg", "sh_bg", "sh_wu", "sh_bu", "sh_wd", "sh_bd",
             "r_wg", "r_bg", "r_wu", "r_bu", "r_wd", "r_bd"]


def _prep(cfg, inputs):
    args = [np.asarray(inputs[k]) for k in _IN_ORDER]
    return host_prep(cfg, *args)


def kernel(**inputs):
    cfg = FULL
    in_maps = _prep(cfg, inputs)
    if "nc" not in _CACHE:
        _CACHE["nc"] = build_nc(cfg)
    nc = _CACHE["nc"]
    res = bass_utils.run_bass_kernel_spmd(nc, in_maps, core_ids=list(range(cfg["NCORES"])))
    B, S, D = np.asarray(inputs["src"]).shape
    out = np.empty((B, S, D), np.float32)
    for b in range(B):
        out[b] = res.results[b]["outT"].T
    return out

